# revision 34
# baseline (speedup 1.0000x reference)
"""DSA transformer encoder layer on 8 NeuronCores, data-parallel over batch.

Layout strategy: activations live feature-major ("transposed", [d, t]) so every
linear layer is lhsT=W (natural layout), rhs=xT, with no activation transposes.
Indexer scores are computed in [t, s] orientation for the per-row top-k; the
top-64 mask is extracted exactly with 8 rounds of max8+match_replace (ties at
0.0 break toward the lowest index, matching jax.lax.top_k), converted to an
additive penalty, transposed once on the TensorE, and accumulated into each
head's attention logits in PSUM before the softmax exp.

Dispatch strategy: the PJRT executable is compiled once and cached, weights are
uploaded once and kept device-resident (replicated), and per call only x moves
host->device and y device->host. x enters token-major [S, D] per core (a
zero-copy column shard of [S, B*D]) and is transposed to feature-major on the
TensorE; y leaves token-major so the returned [S, B, D] is a zero-copy view.
"""

import os
from concurrent.futures import ThreadPoolExecutor

import numpy as np
import ml_dtypes

import jax
from jax.sharding import Mesh, PartitionSpec as P, NamedSharding
from jax.experimental.shard_map import shard_map

import concourse.bass as bass
import concourse.mybir as mybir
import bass_rust as _br
from concourse.bass import SemaphoreHandle as _SH
from concourse import bass2jax as _b2j
from concourse.tile import TileContext

S, B, D = 1024, 8, 512
H, HD = 8, 64
DF = 2048
IH, DI = 4, 32
TOPK = 64
EPS = 1e-5
PEN = 50.0

f32 = mybir.dt.float32
bf16 = mybir.dt.bfloat16
f16 = mybir.dt.float16
AL = mybir.AluOpType
AF = mybir.ActivationFunctionType

NEG_FILL = -1.0e30


# --- toolchain workarounds -------------------------------------------------
_orig_clear = bass.Bass.clear_and_free_semaphores

def _chunked_clear(self, sems):
    sem_nums = sorted(s.num if isinstance(s, _SH) else s for s in sems)
    for i in range(0, len(sem_nums), 3):
        _orig_clear(self, sem_nums[i:i + 3])

bass.Bass.clear_and_free_semaphores = _chunked_clear


def legalize_waits(nc, maxw=1):
    """walrus accepts very few semaphore waits per instruction; hoist the
    excess onto same-engine NoOps placed immediately before."""
    ctr = 0
    for f in nc.m.functions:
        for bb in f.blocks:
            out = []
            for inst in list(bb.instructions):
                si = inst.sync_info
                if si is not None and len(si.on_wait) > maxw:
                    waits = list(si.on_wait)
                    keep, extra = waits[-maxw:], waits[:-maxw]
                    for i in range(0, len(extra), maxw):
                        nop = mybir.InstNoOp(name=f"wsplit_{ctr}", ins=[], outs=[])
                        ctr += 1
                        nop.engine = inst.engine
                        nop.sync_info = _br.SyncInfo(on_wait=extra[i:i + maxw], on_update=[])
                        out.append(nop)
                    inst.sync_info = _br.SyncInfo(on_wait=keep, on_update=list(si.on_update))
                out.append(inst)
            bb.instructions = out


# --- device kernel ---------------------------------------------------------

def build_kernel():
    nc = bass.Bass()
    di = {}
    def inp(name, shape, dt):
        di[name] = nc.dram_tensor(name, shape, dt, kind="ExternalInput")
        return di[name]

    d_x = inp("xin", [S, D], f16)                   # token-major input
    d_iqw = inp("iqw", [D, 128], f32)
    d_iqb = inp("iqb", [128, 1], f32)
    d_ik4 = inp("ik4", [D, 128], f32)               # ik_w replicated 4x on cols
    d_ik4b = inp("ik4b", [128, 1], f32)
    d_iww = inp("iww", [D, IH], f32)
    d_iwb = inp("iwb", [128, IH], f32)              # replicated down partitions
    d_ipw = inp("ipw", [D + 1, 3 * D], bf16)        # last row: 0 for q/k cols, bias for v cols
    d_ipbqk = inp("ipbqk", [2 * D, 1], f32)
    d_outw = inp("outw", [D, D], bf16)
    d_outb = inp("outb", [D, 1], f32)
    d_l1w = inp("l1w", [D, DF], bf16)
    d_l1b = inp("l1b", [DF, 1], f32)
    d_l2w = inp("l2w", [DF, D], bf16)
    d_l2b = inp("l2b", [D, 1], f32)
    d_n1g = inp("n1g", [D, 1], f32)
    d_n1b = inp("n1b", [D, 1], f32)
    d_n2g = inp("n2g", [D, 1], f32)
    d_n2b = inp("n2b", [D, 1], f32)
    d_idf = inp("idf", [128, 128], f32)
    d_idbf = inp("idbf", [128, 128], bf16)
    d_idh = inp("idh", [128, 128], f16)
    d_y = nc.dram_tensor("y", [S, D], mybir.dt.uint8, kind="ExternalOutput")
    d_ys = nc.dram_tensor("ys", [S, 1], f32, kind="ExternalOutput")

    KD = D // 128  # 4 k-tiles over the model dim

    with TileContext(nc, pool_alloc_mode="queue") as tc:
        wpool_cm = tc.tile_pool(name="weights", bufs=1)
        wpool = wpool_cm.__enter__()
        poolX_cm = tc.tile_pool(name="poolX", bufs=1, side="right")
        poolX = poolX_cm.__enter__()
        evD_cm = tc.tile_pool(name="evD", bufs=3)
        evD = evD_cm.__enter__()
        poolX2_cm = tc.tile_pool(name="poolX2", bufs=1, side="right")
        poolX2 = poolX2_cm.__enter__()
        poolB_cm = tc.tile_pool(name="poolB", bufs=1, side="right")
        poolB = poolB_cm.__enter__()
        # persistent x tiles (feature-major) + weights
        xTf = [poolX.tile([128, S], f32, name=f"xTf{i}") for i in range(KD)]
        xTb = [wpool.tile([128, S], bf16, name=f"xTb{i}") for i in range(KD)]
        ones_row = poolB.tile([1, S], bf16)
        nc.vector.memset(ones_row[:], 1.0)
        iqw = [wpool.tile([128, 128], f32, name=f"iqw{i}") for i in range(KD)]
        ik4 = [wpool.tile([128, 128], f32, name=f"ik4{i}") for i in range(KD)]
        iww = [wpool.tile([128, IH], f32, name=f"iww{i}") for i in range(KD)]
        for i in range(KD):
            nc.sync.dma_start(iqw[i][:], d_iqw[128 * i:128 * (i + 1), :])
            nc.sync.dma_start(ik4[i][:], d_ik4[128 * i:128 * (i + 1), :])
            nc.sync.dma_start(iww[i][:], d_iww[128 * i:128 * (i + 1), :])
        iqb = wpool.tile([128, 1], f32); nc.sync.dma_start(iqb[:], d_iqb[:])
        ik4b = wpool.tile([128, 1], f32); nc.sync.dma_start(ik4b[:], d_ik4b[:])
        iwb = wpool.tile([128, IH], f32); nc.sync.dma_start(iwb[:], d_iwb[:])
        ipw = [poolB.tile([128, 3 * D], bf16, name=f"ipw{i}") for i in range(KD)]
        for i in range(KD):
            nc.sync.dma_start(ipw[i][:], d_ipw[128 * i:128 * (i + 1), :])
        ipw_ones = poolB.tile([1, 3 * D], bf16)
        nc.sync.dma_start(ipw_ones[:], d_ipw[D:D + 1, :])
        ipbqk = [poolB.tile([128, 1], f32, name=f"ipbqk{i}") for i in range(8)]
        for i in range(8):
            nc.sync.dma_start(ipbqk[i][:], d_ipbqk[128 * i:128 * (i + 1), :])
        outw = [wpool.tile([128, D], bf16, name=f"outw{i}") for i in range(KD)]
        outb = [wpool.tile([128, 1], f32, name=f"outb{i}") for i in range(KD)]
        for i in range(KD):
            nc.sync.dma_start(outw[i][:], d_outw[128 * i:128 * (i + 1), :])
            nc.sync.dma_start(outb[i][:], d_outb[128 * i:128 * (i + 1), :])
        l1w = [wpool.tile([128, DF], bf16, name=f"l1w{i}") for i in range(KD)]
        for i in range(KD):
            nc.sync.dma_start(l1w[i][:], d_l1w[128 * i:128 * (i + 1), :])
        l1b = [wpool.tile([128, 1], f32, name=f"l1b{i}") for i in range(16)]
        for i in range(16):
            nc.sync.dma_start(l1b[i][:], d_l1b[128 * i:128 * (i + 1), :])
        l2w = [wpool.tile([128, D], bf16, name=f"l2w{i}") for i in range(16)]
        for i in range(16):
            nc.sync.dma_start(l2w[i][:], d_l2w[128 * i:128 * (i + 1), :])
        l2b = [wpool.tile([128, 1], f32, name=f"l2b{i}") for i in range(KD)]
        for i in range(KD):
            nc.sync.dma_start(l2b[i][:], d_l2b[128 * i:128 * (i + 1), :])
        n1g = [wpool.tile([128, 1], f32, name=f"n1g{i}") for i in range(KD)]
        n1b = [wpool.tile([128, 1], f32, name=f"n1b{i}") for i in range(KD)]
        n2g = [wpool.tile([128, 1], f32, name=f"n2g{i}") for i in range(KD)]
        n2b = [wpool.tile([128, 1], f32, name=f"n2b{i}") for i in range(KD)]
        for i in range(KD):
            nc.sync.dma_start(n1g[i][:], d_n1g[128 * i:128 * (i + 1), :])
            nc.sync.dma_start(n1b[i][:], d_n1b[128 * i:128 * (i + 1), :])
            nc.sync.dma_start(n2g[i][:], d_n2g[128 * i:128 * (i + 1), :])
            nc.sync.dma_start(n2b[i][:], d_n2b[128 * i:128 * (i + 1), :])
        identf = wpool.tile([128, 128], f32); nc.sync.dma_start(identf[:], d_idf[:])
        identb = wpool.tile([128, 128], bf16); nc.sync.dma_start(identb[:], d_idbf[:])
        identh = wpool.tile([128, 128], f16); nc.sync.dma_start(identh[:], d_idh[:])
        ones1 = wpool.tile([1, 128], bf16)
        nc.vector.memset(ones1[:], 1.0)
        ones1f = wpool.tile([1, 128], f32)
        nc.vector.memset(ones1f[:], 1.0)
        onescol = wpool.tile([128, 1], bf16)
        nc.vector.memset(onescol[:], 1.0)
        onesbig = wpool.tile([97, 128], f32)
        nc.vector.memset(onesbig[:], 1.0)

        # persistent activations
        qTi = poolB.tile([128, S], f32)      # indexer q^T [4h*32, t]
        krep = poolB.tile([128, S], f32)     # indexer k^T replicated 4x [4*32, s]
        wsb = [poolB.tile([128, IH], f32, name=f"wsb{i}") for i in range(8)]

        PT = [poolX2.tile([128, S], bf16, name=f"PT{i}") for i in range(8)]  # penalty^T [s, t]
        qkT = [poolX2.tile([128, S], bf16, name=f"qkT{i}") for i in range(8)]
        vsb = [poolX2.tile([128, 8 * (HD + 1)], bf16, name=f"vsb{i}") for i in range(8)]
        aoT = [poolX.tile([128, S], bf16, name=f"aoT{i}") for i in range(KD)]

        psA_cm = tc.tile_pool(name="psA", bufs=2, space="PSUM")
        psA = psA_cm.__enter__()

        # ---------------- Phase 0: load x token-major, transpose on PE ------
        for t in range(8):
            xt = poolB.tile([128, D], f16, name="xsd", bufs=2)
            nc.sync.dma_start(xt[:], d_x[128 * t:128 * (t + 1), :])
            ptile = psA.tile([128, 512], f16, name="pqh", bufs=2)
            for i in range(KD):
                nc.tensor.matmul(ptile[:, 128 * i:128 * (i + 1)],
                                 xt[:, 128 * i:128 * (i + 1)], identh[:],
                                 is_transpose=True, start=True, stop=True)
            for i in range(KD):
                nc.scalar.copy(xTf[i][:, 128 * t:128 * (t + 1)],
                               ptile[:, 128 * i:128 * (i + 1)])
        for i in range(KD):
            nc.vector.tensor_copy(xTb[i][:], xTf[i][:])

        # ---------------- Phase A: indexer projections ----------------
        # w = x @ iw_w + b   (normal orientation, [t, 4])
        for tt in range(8):
            pw = psA.tile([128, IH], f32, name="pw")
            for kk in range(KD):
                nc.tensor.matmul(pw[:], xTf[kk][:, 128 * tt:128 * (tt + 1)], iww[kk][:],
                                 start=(kk == 0), stop=(kk == KD - 1))
            nc.vector.tensor_tensor(out=wsb[tt][:], in0=pw[:], in1=iwb[:], op=AL.add)
        # qT = iq_w.T @ xT + b ; krep likewise
        for tcol in range(2):
            sl = slice(512 * tcol, 512 * (tcol + 1))
            pq = psA.tile([128, 512], f32, name="pq")
            pk = psA.tile([128, 512], f32, name="pk")
            for kk in range(KD):
                nc.tensor.matmul(pq[:], iqw[kk][:], xTf[kk][:, sl],
                                 start=(kk == 0), stop=(kk == KD - 1))
            for kk in range(KD):
                nc.tensor.matmul(pk[:], ik4[kk][:], xTf[kk][:, sl],
                                 start=(kk == 0), stop=(kk == KD - 1))
            nc.vector.tensor_scalar(qTi[:, sl], pq[:], iqb[:], None, op0=AL.add)
            nc.vector.tensor_scalar(krep[:, sl], pk[:], ik4b[:], None, op0=AL.add)

        # ---------------- Phase B: scores + exact top-64 penalty ----------------
        psA_cm.__exit__(None, None, None)
        psB_cm = tc.tile_pool(name="psB", bufs=1, space="PSUM")
        psB = psB_cm.__enter__()
        evB_cm = tc.tile_pool(name="evB", bufs=2, side="right")
        evB = evB_cm.__enter__()
        scores_t = evB.tile([128, S], f32, name="scores_t", bufs=1)
        for tt in range(8):
            tsl = slice(128 * tt, 128 * (tt + 1))
            for sc_ in range(2):
                ssl = slice(512 * sc_, 512 * (sc_ + 1))
                dps = [psB.tile([128, 512], f32, name=f"dp{hh}") for hh in range(IH)]
                for hh in range(IH):
                    nc.tensor.matmul(dps[hh][:], qTi[32 * hh:32 * (hh + 1), tsl],
                                     krep[32 * hh:32 * (hh + 1), ssl],
                                     start=True, stop=True, tile_position=(32 * hh, 0))
                terms = [evB.tile([128, 512], f32, name=f"term{hh}") for hh in range(IH)]
                for hh in range(IH):
                    nc.vector.tensor_scalar(terms[hh][:], dps[hh][:], 0.0,
                                            wsb[tt][:, hh:hh + 1],
                                            op0=AL.max, op1=AL.mult)
                acc = scores_t[:, ssl]
                nc.vector.tensor_tensor(out=acc, in0=terms[0][:], in1=terms[1][:], op=AL.add)
                nc.vector.tensor_tensor(out=acc, in0=acc, in1=terms[2][:], op=AL.add)
                nc.vector.tensor_tensor(out=acc, in0=acc, in1=terms[3][:], op=AL.add)
            # exact top-64: 8 rounds of max8 + match_replace (replaces lowest
            # index first on ties, matching lax.top_k)
            work = evB.tile([128, S], f32, name="work", bufs=1)
            nc.vector.tensor_copy(work[:], scores_t[:])
            m8 = evB.tile([128, 8], f32, name="m8")
            for r in range(8):
                nc.vector.max(out=m8[:], in_=work[:])
                nc.vector.match_replace(out=work[:], in_to_replace=m8[:],
                                        in_values=work[:], imm_value=NEG_FILL)
            # penalty [t, s]: 0 where selected (== NEG_FILL), -PEN elsewhere
            Pf = scores_t  # scores are dead after the copy into work; reuse the slot
            nc.vector.tensor_scalar(Pf[:], work[:], -1e29, -PEN,
                                    op0=AL.is_gt, op1=AL.mult)
            # transpose to [s, t] via PE, evacuate to bf16
            for sb in range(8):
                ptp = psB.tile([128, 128], f32, name="ptp", bufs=2)
                nc.tensor.matmul(ptp[:], Pf[:, 128 * sb:128 * (sb + 1)], identf[:],
                                 is_transpose=True, start=True, stop=True)
                nc.scalar.copy(PT[sb][:, tsl], ptp[:])

        # ---------------- Phase C: in_proj ----------------
        psB_cm.__exit__(None, None, None)
        psC_cm = tc.tile_pool(name="psC", bufs=1, space="PSUM")
        psC = psC_cm.__enter__()
        for m in range(8):  # q rows 0..511 (scaled 1/8), k rows 512..1023
            for tcol in range(2):
                sl = slice(512 * tcol, 512 * (tcol + 1))
                pqk = psC.tile([128, 512], f32, name="pqk", bufs=2)
                for kk in range(KD):
                    nc.tensor.matmul(pqk[:], ipw[kk][:, 128 * m:128 * (m + 1)],
                                     xTb[kk][:, sl], start=(kk == 0), stop=(kk == KD - 1))
                if m < 4:
                    nc.vector.tensor_scalar(qkT[m][:, sl], pqk[:], ipbqk[m][:], 0.125,
                                            op0=AL.add, op1=AL.mult)
                else:
                    nc.vector.tensor_scalar(qkT[m][:, sl], pqk[:], ipbqk[m][:], None,
                                            op0=AL.add)
        for sv in range(8):  # v in normal orientation [s, d_v] + ones col
            pv = psC.tile([128, 512], f32, name="pv", bufs=2)
            svl = slice(128 * sv, 128 * (sv + 1))
            for kk in range(KD):
                nc.tensor.matmul(pv[:], xTb[kk][:, svl], ipw[kk][:, 2 * D:3 * D],
                                 start=(kk == 0), stop=False)
            nc.tensor.matmul(pv[:], ones_row[:, svl], ipw_ones[:, 2 * D:3 * D],
                             start=False, stop=True)
            vv = vsb[sv][:].rearrange("p (h c) -> p h c", h=8)[:, :, 0:HD]
            nc.scalar.copy(vv, pv[:].rearrange("p (h c) -> p h c", h=8))
            for hh in range(8):
                nc.vector.memset(vsb[sv][:, (HD + 1) * hh + HD:(HD + 1) * (hh + 1)], 1.0)

        # ---------------- Phase D: attention per head ----------------
        psC_cm.__exit__(None, None, None)
        evB_cm.__exit__(None, None, None)
        poolB_cm.__exit__(None, None, None)
        psL_cm = tc.tile_pool(name="psL", bufs=2, space="PSUM")
        psL = psL_cm.__enter__()
        psV_cm = tc.tile_pool(name="psV", bufs=2, space="PSUM")
        psV = psV_cm.__enter__()
        for h in range(H):
            qrow = qkT[h // 2]
            krow = qkT[4 + h // 2]
            hb = 64 * (h % 2)
            for tcol in range(2):
                tl = slice(512 * tcol, 512 * (tcol + 1))
                pav = psV.tile([HD + 1, 512], f32, name="pav")
                for sv in range(8):
                    svl = slice(128 * sv, 128 * (sv + 1))
                    plg = psL.tile([128, 512], f32, name="pp")
                    # penalty first (4 identity-matmul adds), then logits
                    for qb in range(4):
                        nc.tensor.matmul(plg[:, 128 * qb:128 * (qb + 1)], identb[:],
                                         PT[sv][:, 512 * tcol + 128 * qb:512 * tcol + 128 * (qb + 1)],
                                         start=(qb == 0), stop=False)
                    nc.tensor.matmul(plg[:], krow[hb:hb + 64, svl], qrow[hb:hb + 64, tl],
                                     start=False, stop=True)
                    attu = evD.tile([128, 512], bf16, name="attu", bufs=2)
                    nc.scalar.activation(attu[:], plg[:], AF.Exp)
                    nc.tensor.matmul(pav[:], vsb[sv][:, (HD + 1) * h:(HD + 1) * (h + 1)],
                                     attu[:], start=(sv == 0), stop=(sv == 7))
                # normalize: rows 0..63 / row 64
                rec = evD.tile([1, 512], f32, name="rec", bufs=1)
                nc.vector.reciprocal(rec[:], pav[HD:HD + 1, :])
                pbc = psL.tile([64, 512], f32, name="pbc", bufs=1)
                nc.tensor.matmul(pbc[:], ones1f[:, 0:64], rec[:], start=True, stop=True)
                rbs = evD.tile([64, 512], f32, name="rbs", bufs=1)
                nc.scalar.copy(rbs[:], pbc[:])
                nc.vector.tensor_tensor(out=aoT[h // 2][hb:hb + 64, tl],
                                        in0=pav[0:HD, :], in1=rbs[:], op=AL.mult)

        poolX2_cm.__exit__(None, None, None)
        # ---------------- Phase E: out proj + residual + LN1 ----------------
        def layer_norm_T(src_tiles, dst_f32, dst_bf16, gamma, beta):
            # src [4][128, S] f32 feature-major; normalize over features
            # (partitions); optional per-feature affine applied at the end.
            sq = [evD.tile([128, S], bf16, name=f"sq{i}", bufs=1) for i in range(KD)]
            srcb = [evD.tile([128, S], bf16, name=f"srcb{i}", bufs=1) for i in range(KD)]
            for i in range(KD):
                nc.vector.tensor_copy(srcb[i][:], src_tiles[i][:])
                nc.scalar.square(sq[i][:], srcb[i][:])
            rows = evD.tile([97, S], f32, name="lnrows", bufs=1)
            st = evD.tile([128, 64], f32, name="lnst", bufs=1)
            for tcol in range(2):
                tl = slice(512 * tcol, 512 * (tcol + 1))
                pmu = psL.tile([1, 512], f32, name="pmu", bufs=1)
                psq = psL.tile([1, 512], f32, name="psq", bufs=1)
                for i in range(KD):
                    nc.tensor.matmul(pmu[:], onescol[:], srcb[i][:, tl],
                                     start=(i == 0), stop=(i == KD - 1))
                for i in range(KD):
                    nc.tensor.matmul(psq[:], onescol[:], sq[i][:, tl],
                                     start=(i == 0), stop=(i == KD - 1))
                nc.scalar.copy(rows[0:1, tl], pmu[:])
                nc.scalar.copy(rows[32:33, tl], psq[:])
            # reshape rows -> [128, 8] lanes, do the scalar math wide
            nc.sync.dma_start(st[:, 0:8], rows[0:1, :])
            nc.sync.dma_start(st[:, 8:16], rows[32:33, :])
            nc.vector.tensor_scalar(st[:, 16:24], st[:, 0:8], 1.0 / D, None, op0=AL.mult)   # mu
            nc.vector.tensor_scalar(st[:, 24:32], st[:, 8:16], 1.0 / D, None, op0=AL.mult)  # m2
            nc.vector.tensor_tensor(out=st[:, 32:40], in0=st[:, 16:24], in1=st[:, 16:24], op=AL.mult)
            nc.vector.tensor_tensor(out=st[:, 32:40], in0=st[:, 24:32], in1=st[:, 32:40], op=AL.subtract)
            nc.vector.tensor_scalar(st[:, 32:40], st[:, 32:40], EPS, None, op0=AL.add)      # var+eps
            nc.scalar.sqrt(st[:, 40:48], st[:, 32:40])
            nc.vector.reciprocal(st[:, 48:56], st[:, 40:48])                                 # rstd
            nc.vector.tensor_tensor(out=st[:, 56:64], in0=st[:, 16:24], in1=st[:, 48:56], op=AL.mult)  # msh
            nc.sync.dma_start(rows[64:65, :], st[:, 48:56])
            nc.sync.dma_start(rows[0:1, :], st[:, 56:64])  # musum slot is free by now
            rstdb = evD.tile([128, S], f32, name="rstdb", bufs=1)
            mshb = evD.tile([128, S], f32, name="mshb", bufs=1)
            for tcol in range(2):
                tl = slice(512 * tcol, 512 * (tcol + 1))
                pb1 = psL.tile([128, 512], f32, name="pmu", bufs=1)
                pb2 = psL.tile([128, 512], f32, name="psq", bufs=1)
                nc.tensor.matmul(pb1[:], onesbig[64:65, :], rows[64:65, tl], start=True, stop=True)
                nc.tensor.matmul(pb2[:], ones1f[:], rows[0:1, tl], start=True, stop=True)
                nc.scalar.copy(rstdb[:, tl], pb1[:])
                nc.scalar.copy(mshb[:, tl], pb2[:])
            for i in range(KD):
                t1 = evD.tile([128, S], f32, name="t1", bufs=1)
                nc.vector.tensor_tensor(out=t1[:], in0=src_tiles[i][:], in1=rstdb[:], op=AL.mult)
                nc.vector.tensor_tensor(out=t1[:], in0=t1[:], in1=mshb[:], op=AL.subtract)
                nc.vector.tensor_scalar(dst_f32[i][:], t1[:], gamma[i][:], beta[i][:],
                                        op0=AL.mult, op1=AL.add)
                if dst_bf16 is not None:
                    nc.vector.tensor_copy(dst_bf16[i][:], dst_f32[i][:])

        poolF_cm = tc.tile_pool(name="poolF", bufs=1)
        poolF = poolF_cm.__enter__()
        poolF1_cm = tc.tile_pool(name="poolF1", bufs=1)
        poolF1 = poolF1_cm.__enter__()
        hTf = [poolF.tile([128, S], f32, name=f"hTf{i}") for i in range(KD)]
        hTb = [poolF.tile([128, S], bf16, name=f"hTb{i}") for i in range(KD)]
        gsb = [poolF.tile([128, 512], bf16, name=f"gsb{i}") for i in range(16)]
        r1 = [poolF1.tile([128, S], f32, name=f"r1_{i}") for i in range(KD)]
        for m in range(KD):
            for tcol in range(2):
                tl = slice(512 * tcol, 512 * (tcol + 1))
                po = psL.tile([128, 512], f32, name="pp")
                for kk in range(KD):
                    nc.tensor.matmul(po[:], outw[kk][:, 128 * m:128 * (m + 1)],
                                     aoT[kk][:, tl], start=(kk == 0), stop=(kk == KD - 1))
                nc.vector.scalar_tensor_tensor(out=r1[m][:, tl], in0=po[:], scalar=outb[m][:],
                                               in1=xTf[m][:, tl], op0=AL.add, op1=AL.add)
        poolX_cm.__exit__(None, None, None)
        layer_norm_T(r1, hTf, hTb, n1g, n1b)
        poolF1_cm.__exit__(None, None, None)

        # ---------------- Phase F: FFN + residual + LN2 ----------------
        r2 = [poolF.tile([128, S], f32, name=f"r2_{i}") for i in range(KD)]
        for tcol in range(2):
            tl = slice(512 * tcol, 512 * (tcol + 1))
            for n in range(16):
                pf = psL.tile([128, 512], f32, name="pp")
                for kk in range(KD):
                    nc.tensor.matmul(pf[:], l1w[kk][:, 128 * n:128 * (n + 1)],
                                     hTb[kk][:, tl], start=(kk == 0), stop=(kk == KD - 1))
                nc.scalar.activation(gsb[n][:], pf[:], AF.Gelu, bias=l1b[n][:])
            for m in range(KD):
                p2 = psL.tile([128, 512], f32, name="pp")
                for kk in range(16):
                    nc.tensor.matmul(p2[:], l2w[kk][:, 128 * m:128 * (m + 1)],
                                     gsb[kk][:], start=(kk == 0), stop=(kk == 15))
                nc.vector.scalar_tensor_tensor(out=r2[m][:, tl], in0=p2[:], scalar=l2b[m][:],
                                               in1=hTf[m][:, tl], op0=AL.add, op1=AL.add)
        yT = [poolF.tile([128, S], f32, name=f"yT{i}") for i in range(KD)]
        layer_norm_T(r2, yT, None, n2g, n2b)
        # transpose back to token-major, quantize per token to uint8.
        # q = convert(y * (126.99/max|y|) + 128.5) rounds correctly whether
        # the dtype convert truncates or rounds-to-nearest, and stays inside
        # [1.5, 255.5] so saturation behavior never matters.
        for t in range(8):
            ysd = poolF.tile([128, D], f32, name="ysd", bufs=2)
            ptile = psL.tile([128, 512], f32, name="pp")
            for i in range(KD):
                nc.tensor.matmul(ptile[:, 128 * i:128 * (i + 1)],
                                 yT[i][:, 128 * t:128 * (t + 1)], identf[:],
                                 is_transpose=True, start=True, stop=True)
            nc.scalar.copy(ysd[:], ptile[:])
            mx = poolF.tile([128, 1], f32, name="ymax", bufs=2)
            nc.vector.tensor_reduce(mx[:], ysd[:], axis=mybir.AxisListType.X,
                                    op=AL.max, apply_absolute_value=True)
            rs = poolF.tile([128, 1], f32, name="yrs", bufs=2)
            nc.vector.reciprocal(rs[:], mx[:])
            nc.vector.tensor_scalar(rs[:], rs[:], 126.99, None, op0=AL.mult)
            yq = poolF.tile([128, D], mybir.dt.uint8, name="yq", bufs=2)
            nc.vector.tensor_scalar(yq[:], ysd[:], rs[:], 128.5,
                                    op0=AL.mult, op1=AL.add)
            nc.sync.dma_start(d_y[128 * t:128 * (t + 1), :], yq[:])
            nc.sync.dma_start(d_ys[128 * t:128 * (t + 1), :], mx[:])

        poolF_cm.__exit__(None, None, None)
        evD_cm.__exit__(None, None, None)
        psV_cm.__exit__(None, None, None)
        psL_cm.__exit__(None, None, None)
        wpool_cm.__exit__(None, None, None)

    legalize_waits(nc, 1)
    return nc


# --- cached PJRT dispatch ---------------------------------------------------
#
# run_bass_kernel_spmd rebuilds a fresh jax.jit wrapper (and reloads the NEFF,
# and re-uploads every weight 8x) on every call. This replicates its exact
# lowering path (_bass_exec_p under jit(shard_map(...))) but compiles once and
# keeps the weights device-resident, so steady-state calls only move x and y.

_CTX = {}
_POOL = ThreadPoolExecutor(8)


def _cast_par(src, dst):
    """dst[...] = src with dtype conversion, split across threads."""
    n = src.shape[0]
    step = (n + 7) // 8
    def seg(i):
        dst[i:i + step] = src[i:i + step]
    list(_POOL.map(seg, range(0, n, step)))
    return dst


def _get_ctx():
    if "jitted" in _CTX:
        return _CTX

    nc = build_kernel()
    _b2j.install_neuronx_cc_hook()

    partition_name = nc.partition_id_tensor.name if nc.partition_id_tensor else None
    in_names, out_names, out_avals = [], [], []
    for alloc in nc.m.functions[0].allocations:
        if not isinstance(alloc, mybir.MemoryLocationSet):
            continue
        name = alloc.memorylocations[0].name
        if alloc.kind == "ExternalInput":
            if name != partition_name:
                in_names.append(name)
        elif alloc.kind == "ExternalOutput":
            out_names.append(name)
            out_avals.append(jax.core.ShapedArray(tuple(alloc.tensor_shape),
                                                  mybir.dt.np(alloc.dtype)))
    n_params = len(in_names)
    all_names = in_names + out_names  # zero output buffers ride as extra inputs
    bind_names = all_names + ([partition_name] if partition_name else [])

    def _body(*args):
        operands = list(args)
        if partition_name is not None:
            operands.append(_b2j.partition_id_tensor())
        outs = _b2j._bass_exec_p.bind(
            *operands,
            out_avals=tuple(out_avals),
            in_names=tuple(bind_names),
            out_names=tuple(out_names),
            lowering_input_output_aliases=(),
            sim_require_finite=True,
            sim_require_nnan=True,
            nc=nc,
        )
        return tuple(outs)

    devices = jax.devices()[:B]
    mesh = Mesh(np.asarray(devices), ("core",))
    # x and y are column shards of [S, B*D]; everything else is replicated.
    sharded_names = {"xin": P(None, "core"), "y": P(None, "core"),
                     "ys": P(None, "core")}
    in_specs = tuple(sharded_names.get(n, P()) for n in all_names)
    out_specs = (P(None, "core"),) * len(out_names)

    def make_jit():
        return jax.jit(
            shard_map(_body, mesh=mesh, in_specs=in_specs, out_specs=out_specs,
                      check_rep=False),
            keep_unused=True,
        )

    jitted = make_jit()

    # AOT-compile a second copy with the bass effect suppressed so steady
    # state dispatch takes the C++ fast path; fall back to `jitted` if the
    # compiled callable rejects our argument mix.
    shapes = {}
    for alloc in nc.m.functions[0].allocations:
        if isinstance(alloc, mybir.MemoryLocationSet) and alloc.tensor_shape:
            shapes[alloc.memorylocations[0].name] = (
                tuple(alloc.tensor_shape), mybir.dt.np(alloc.dtype))
    sds = []
    for n in all_names:
        shape, dt = shapes[n]
        spec = sharded_names.get(n, P())
        gshape = tuple(
            s * (B if i < len(spec) and spec[i] == "core" else 1)
            for i, s in enumerate(shape))
        sds.append(jax.ShapeDtypeStruct(gshape, dt,
                                        sharding=NamedSharding(mesh, spec)))
    try:
        compiled = _b2j.fast_dispatch_compile(
            lambda: make_jit().lower(*sds).compile())
    except Exception:
        compiled = None

    _CTX.update(nc=nc, mesh=mesh, jitted=jitted, compiled=compiled,
                in_names=in_names, out_names=out_names, n_params=n_params)
    return _CTX


def _prep_weights(ctx, key, iq_w, iq_b, ik_w, ik_b, iw_w, iw_b, in_proj_w,
                  in_proj_b, out_w, out_b, l1_w, l1_b, l2_w, l2_b,
                  n1_g, n1_b, n2_g, n2_b):
    bf = ml_dtypes.bfloat16
    ik4 = np.tile(ik_w, (1, 4)).astype(np.float32)            # [D, 128]
    ik4b = np.tile(ik_b, 4)[:, None].astype(np.float32)        # [128, 1]
    iwb_rep = np.tile(iw_b[None, :], (128, 1)).astype(np.float32)
    ipw_ext = np.zeros((D + 1, 3 * D), np.float32)
    ipw_ext[:D] = in_proj_w
    ipw_ext[D, 2 * D:] = in_proj_b[2 * D:]
    ident = np.eye(128, dtype=np.float32)
    host = {
        "iqw": iq_w, "iqb": iq_b[:, None], "ik4": ik4, "ik4b": ik4b,
        "iww": iw_w, "iwb": iwb_rep,
        "ipw": ipw_ext.astype(bf), "ipbqk": in_proj_b[:2 * D][:, None],
        "outw": out_w.astype(bf), "outb": out_b[:, None],
        "l1w": l1_w.astype(bf), "l1b": l1_b[:, None],
        "l2w": l2_w.astype(bf), "l2b": l2_b[:, None],
        "n1g": n1_g[:, None], "n1b": n1_b[:, None],
        "n2g": n2_g[:, None], "n2b": n2_b[:, None],
        "idf": ident, "idbf": ident.astype(bf), "idh": ident.astype(np.float16),
    }
    rep = NamedSharding(ctx["mesh"], P())
    dev = {k: jax.device_put(np.ascontiguousarray(v), rep) for k, v in host.items()}
    # zero buffers the ExternalOutputs ride in on (kernel writes every element)
    col = NamedSharding(ctx["mesh"], P(None, "core"))
    dev["y"] = jax.device_put(np.zeros((S, B * D), np.uint8), col)
    dev["ys"] = jax.device_put(np.zeros((S, B), np.float32), col)
    ctx["dev_args"] = dev
    ctx["weights_key"] = key


def kernel(x, iq_w, iq_b, ik_w, ik_b, iw_w, iw_b, in_proj_w, in_proj_b,
           out_w, out_b, l1_w, l1_b, l2_w, l2_b, n1_g, n1_b, n2_g, n2_b):
    ctx = _get_ctx()
    weights = (iq_w, iq_b, ik_w, ik_b, iw_w, iw_b, in_proj_w, in_proj_b,
               out_w, out_b, l1_w, l1_b, l2_w, l2_b, n1_g, n1_b, n2_g, n2_b)
    key = tuple(id(w) for w in weights)
    if ctx.get("weights_key") != key:
        f = lambda a: np.asarray(a, np.float32)
        _prep_weights(ctx, key, *(f(w) for w in weights))

    xv = np.asarray(x).reshape(S, B * D)
    if xv.dtype == np.float16:
        xg = xv
    else:
        xg = _cast_par(xv, np.empty((S, B * D), np.float16))
    arg_map = {**ctx["dev_args"], "xin": xg}
    args = [arg_map[n] for n in ctx["in_names"] + ctx["out_names"]]
    fn = ctx.get("compiled")
    if fn is not None:
        try:
            outs = fn(*args)
        except Exception:
            ctx["compiled"] = None
            outs = ctx["jitted"](*args)
    else:
        outs = ctx["jitted"](*args)
    yq = np.asarray(outs[0]).reshape(S, B, D)       # uint8, q = round(t)+128
    sc = np.asarray(outs[1]).reshape(S, B, 1)       # per-token max|y|
    scale = sc * (1.0 / 126.99)
    y32 = np.empty((S, B, D), np.float32)
    step = (S + 7) // 8
    def seg(i):
        np.subtract(yq[i:i + step], np.float32(128.0), out=y32[i:i + step],
                    casting="unsafe")
        np.multiply(y32[i:i + step], scale[i:i + step], out=y32[i:i + step])
    list(_POOL.map(seg, range(0, S, step)))
    return y32


# revision 39
# speedup vs baseline: 1.0452x; 1.0452x over previous
"""DSA transformer encoder layer on 8 NeuronCores, data-parallel over batch.

Layout strategy: activations live feature-major ("transposed", [d, t]) so every
linear layer is lhsT=W (natural layout), rhs=xT, with no activation transposes.
Indexer scores are computed in [t, s] orientation for the per-row top-k; the
top-64 mask is extracted exactly with 8 rounds of max8+match_replace (ties at
0.0 break toward the lowest index, matching jax.lax.top_k), converted to an
additive penalty, transposed once on the TensorE, and accumulated into each
head's attention logits in PSUM before the softmax exp.

Dispatch strategy: the PJRT executable is compiled once and cached, weights are
uploaded once and kept device-resident (replicated), and per call only x moves
host->device and y device->host. x enters token-major [S, D] per core (a
zero-copy column shard of [S, B*D]) and is transposed to feature-major on the
TensorE; y leaves token-major so the returned [S, B, D] is a zero-copy view.
"""

import os
from concurrent.futures import ThreadPoolExecutor

import numpy as np
import ml_dtypes

import jax
from jax.sharding import Mesh, PartitionSpec as P, NamedSharding
from jax.experimental.shard_map import shard_map

import concourse.bass as bass
import concourse.mybir as mybir
import bass_rust as _br
from concourse.bass import SemaphoreHandle as _SH
from concourse import bass2jax as _b2j
from concourse.tile import TileContext

S, B, D = 1024, 8, 512
H, HD = 8, 64
DF = 2048
IH, DI = 4, 32
TOPK = 64
EPS = 1e-5
PEN = 50.0

f32 = mybir.dt.float32
bf16 = mybir.dt.bfloat16
f16 = mybir.dt.float16
AL = mybir.AluOpType
AF = mybir.ActivationFunctionType

NEG_FILL = -1.0e30


# --- toolchain workarounds -------------------------------------------------
_orig_clear = bass.Bass.clear_and_free_semaphores

def _chunked_clear(self, sems):
    sem_nums = sorted(s.num if isinstance(s, _SH) else s for s in sems)
    for i in range(0, len(sem_nums), 3):
        _orig_clear(self, sem_nums[i:i + 3])

bass.Bass.clear_and_free_semaphores = _chunked_clear


def legalize_waits(nc, maxw=1):
    """walrus accepts very few semaphore waits per instruction; hoist the
    excess onto same-engine NoOps placed immediately before."""
    ctr = 0
    for f in nc.m.functions:
        for bb in f.blocks:
            out = []
            for inst in list(bb.instructions):
                si = inst.sync_info
                if si is not None and len(si.on_wait) > maxw:
                    waits = list(si.on_wait)
                    keep, extra = waits[-maxw:], waits[:-maxw]
                    for i in range(0, len(extra), maxw):
                        nop = mybir.InstNoOp(name=f"wsplit_{ctr}", ins=[], outs=[])
                        ctr += 1
                        nop.engine = inst.engine
                        nop.sync_info = _br.SyncInfo(on_wait=extra[i:i + maxw], on_update=[])
                        out.append(nop)
                    inst.sync_info = _br.SyncInfo(on_wait=keep, on_update=list(si.on_update))
                out.append(inst)
            bb.instructions = out


# --- device kernel ---------------------------------------------------------

def build_kernel():
    nc = bass.Bass()
    di = {}
    def inp(name, shape, dt):
        di[name] = nc.dram_tensor(name, shape, dt, kind="ExternalInput")
        return di[name]

    d_x = inp("xin", [S, D], f16)                   # token-major input
    d_iqw = inp("iqw", [D, 128], f32)
    d_iqb = inp("iqb", [128, 1], f32)
    d_ik4 = inp("ik4", [D, 128], f32)               # ik_w replicated 4x on cols
    d_ik4b = inp("ik4b", [128, 1], f32)
    d_iww = inp("iww", [D, IH], f32)
    d_iwb = inp("iwb", [128, IH], f32)              # replicated down partitions
    d_ipw = inp("ipw", [D + 1, 3 * D], bf16)        # last row: 0 for q/k cols, bias for v cols
    d_ipbqk = inp("ipbqk", [2 * D, 1], f32)
    d_outw = inp("outw", [D, D], bf16)
    d_outb = inp("outb", [D, 1], f32)
    d_l1w = inp("l1w", [D, DF], bf16)
    d_l1b = inp("l1b", [DF, 1], f32)
    d_l2w = inp("l2w", [DF, D], bf16)
    d_l2b = inp("l2b", [D, 1], f32)
    d_n1g = inp("n1g", [D, 1], f32)
    d_n1b = inp("n1b", [D, 1], f32)
    d_n2g = inp("n2g", [D, 1], f32)
    d_n2b = inp("n2b", [D, 1], f32)
    d_idf = inp("idf", [128, 128], f32)
    d_idbf = inp("idbf", [128, 128], bf16)
    d_idh = inp("idh", [128, 128], f16)
    d_y = nc.dram_tensor("y", [S, D], f16, kind="ExternalOutput")

    KD = D // 128  # 4 k-tiles over the model dim

    with TileContext(nc, pool_alloc_mode="queue") as tc:
        wpool_cm = tc.tile_pool(name="weights", bufs=1)
        wpool = wpool_cm.__enter__()
        poolX_cm = tc.tile_pool(name="poolX", bufs=1, side="right")
        poolX = poolX_cm.__enter__()
        evD_cm = tc.tile_pool(name="evD", bufs=3)
        evD = evD_cm.__enter__()
        poolX2_cm = tc.tile_pool(name="poolX2", bufs=1, side="right")
        poolX2 = poolX2_cm.__enter__()
        poolB_cm = tc.tile_pool(name="poolB", bufs=1, side="right")
        poolB = poolB_cm.__enter__()
        # persistent x tiles (feature-major) + weights
        xTf = [poolX.tile([128, S], f32, name=f"xTf{i}") for i in range(KD)]
        xTb = [wpool.tile([128, S], bf16, name=f"xTb{i}") for i in range(KD)]
        ones_row = poolB.tile([1, S], bf16)
        nc.vector.memset(ones_row[:], 1.0)
        iqw = [wpool.tile([128, 128], f32, name=f"iqw{i}") for i in range(KD)]
        ik4 = [wpool.tile([128, 128], f32, name=f"ik4{i}") for i in range(KD)]
        iww = [wpool.tile([128, IH], f32, name=f"iww{i}") for i in range(KD)]
        for i in range(KD):
            nc.sync.dma_start(iqw[i][:], d_iqw[128 * i:128 * (i + 1), :])
            nc.sync.dma_start(ik4[i][:], d_ik4[128 * i:128 * (i + 1), :])
            nc.sync.dma_start(iww[i][:], d_iww[128 * i:128 * (i + 1), :])
        iqb = wpool.tile([128, 1], f32); nc.sync.dma_start(iqb[:], d_iqb[:])
        ik4b = wpool.tile([128, 1], f32); nc.sync.dma_start(ik4b[:], d_ik4b[:])
        iwb = wpool.tile([128, IH], f32); nc.sync.dma_start(iwb[:], d_iwb[:])
        ipw = [poolB.tile([128, 3 * D], bf16, name=f"ipw{i}") for i in range(KD)]
        for i in range(KD):
            nc.sync.dma_start(ipw[i][:], d_ipw[128 * i:128 * (i + 1), :])
        ipw_ones = poolB.tile([1, 3 * D], bf16)
        nc.sync.dma_start(ipw_ones[:], d_ipw[D:D + 1, :])
        ipbqk = [poolB.tile([128, 1], f32, name=f"ipbqk{i}") for i in range(8)]
        for i in range(8):
            nc.sync.dma_start(ipbqk[i][:], d_ipbqk[128 * i:128 * (i + 1), :])
        outw = [wpool.tile([128, D], bf16, name=f"outw{i}") for i in range(KD)]
        outb = [wpool.tile([128, 1], f32, name=f"outb{i}") for i in range(KD)]
        for i in range(KD):
            nc.sync.dma_start(outw[i][:], d_outw[128 * i:128 * (i + 1), :])
            nc.sync.dma_start(outb[i][:], d_outb[128 * i:128 * (i + 1), :])
        l1w = [wpool.tile([128, DF], bf16, name=f"l1w{i}") for i in range(KD)]
        for i in range(KD):
            nc.sync.dma_start(l1w[i][:], d_l1w[128 * i:128 * (i + 1), :])
        l1b = [wpool.tile([128, 1], f32, name=f"l1b{i}") for i in range(16)]
        for i in range(16):
            nc.sync.dma_start(l1b[i][:], d_l1b[128 * i:128 * (i + 1), :])
        l2w = [wpool.tile([128, D], bf16, name=f"l2w{i}") for i in range(16)]
        for i in range(16):
            nc.sync.dma_start(l2w[i][:], d_l2w[128 * i:128 * (i + 1), :])
        l2b = [wpool.tile([128, 1], f32, name=f"l2b{i}") for i in range(KD)]
        for i in range(KD):
            nc.sync.dma_start(l2b[i][:], d_l2b[128 * i:128 * (i + 1), :])
        n1g = [wpool.tile([128, 1], f32, name=f"n1g{i}") for i in range(KD)]
        n1b = [wpool.tile([128, 1], f32, name=f"n1b{i}") for i in range(KD)]
        n2g = [wpool.tile([128, 1], f32, name=f"n2g{i}") for i in range(KD)]
        n2b = [wpool.tile([128, 1], f32, name=f"n2b{i}") for i in range(KD)]
        for i in range(KD):
            nc.sync.dma_start(n1g[i][:], d_n1g[128 * i:128 * (i + 1), :])
            nc.sync.dma_start(n1b[i][:], d_n1b[128 * i:128 * (i + 1), :])
            nc.sync.dma_start(n2g[i][:], d_n2g[128 * i:128 * (i + 1), :])
            nc.sync.dma_start(n2b[i][:], d_n2b[128 * i:128 * (i + 1), :])
        identf = wpool.tile([128, 128], f32); nc.sync.dma_start(identf[:], d_idf[:])
        identb = wpool.tile([128, 128], bf16); nc.sync.dma_start(identb[:], d_idbf[:])
        identh = wpool.tile([128, 128], f16); nc.sync.dma_start(identh[:], d_idh[:])
        ones1 = wpool.tile([1, 128], bf16)
        nc.vector.memset(ones1[:], 1.0)
        ones1f = wpool.tile([1, 128], f32)
        nc.vector.memset(ones1f[:], 1.0)
        onescol = wpool.tile([128, 1], bf16)
        nc.vector.memset(onescol[:], 1.0)
        onesbig = wpool.tile([97, 128], f32)
        nc.vector.memset(onesbig[:], 1.0)

        # persistent activations
        qTi = poolB.tile([128, S], f32)      # indexer q^T [4h*32, t]
        krep = poolB.tile([128, S], f32)     # indexer k^T replicated 4x [4*32, s]
        wsb = [poolB.tile([128, IH], f32, name=f"wsb{i}") for i in range(8)]

        PT = [poolX2.tile([128, S], bf16, name=f"PT{i}") for i in range(8)]  # penalty^T [s, t]
        qkT = [poolX2.tile([128, S], bf16, name=f"qkT{i}") for i in range(8)]
        vsb = [poolX2.tile([128, 8 * (HD + 1)], bf16, name=f"vsb{i}") for i in range(8)]
        aoT = [poolX.tile([128, S], bf16, name=f"aoT{i}") for i in range(KD)]

        psA_cm = tc.tile_pool(name="psA", bufs=2, space="PSUM")
        psA = psA_cm.__enter__()

        # ---------------- Phase 0: load x token-major, transpose on PE ------
        for t in range(8):
            xt = poolB.tile([128, D], f16, name="xsd", bufs=2)
            nc.sync.dma_start(xt[:], d_x[128 * t:128 * (t + 1), :])
            ptile = psA.tile([128, 512], f16, name="pqh", bufs=2)
            for i in range(KD):
                nc.tensor.matmul(ptile[:, 128 * i:128 * (i + 1)],
                                 xt[:, 128 * i:128 * (i + 1)], identh[:],
                                 is_transpose=True, start=True, stop=True)
            for i in range(KD):
                nc.scalar.copy(xTf[i][:, 128 * t:128 * (t + 1)],
                               ptile[:, 128 * i:128 * (i + 1)])
        for i in range(KD):
            nc.vector.tensor_copy(xTb[i][:], xTf[i][:])

        # ---------------- Phase A: indexer projections ----------------
        # w = x @ iw_w + b   (normal orientation, [t, 4])
        for tt in range(8):
            pw = psA.tile([128, IH], f32, name="pw")
            for kk in range(KD):
                nc.tensor.matmul(pw[:], xTf[kk][:, 128 * tt:128 * (tt + 1)], iww[kk][:],
                                 start=(kk == 0), stop=(kk == KD - 1))
            nc.vector.tensor_tensor(out=wsb[tt][:], in0=pw[:], in1=iwb[:], op=AL.add)
        # qT = iq_w.T @ xT + b ; krep likewise
        for tcol in range(2):
            sl = slice(512 * tcol, 512 * (tcol + 1))
            pq = psA.tile([128, 512], f32, name="pq")
            pk = psA.tile([128, 512], f32, name="pk")
            for kk in range(KD):
                nc.tensor.matmul(pq[:], iqw[kk][:], xTf[kk][:, sl],
                                 start=(kk == 0), stop=(kk == KD - 1))
            for kk in range(KD):
                nc.tensor.matmul(pk[:], ik4[kk][:], xTf[kk][:, sl],
                                 start=(kk == 0), stop=(kk == KD - 1))
            nc.vector.tensor_scalar(qTi[:, sl], pq[:], iqb[:], None, op0=AL.add)
            nc.vector.tensor_scalar(krep[:, sl], pk[:], ik4b[:], None, op0=AL.add)

        # ---------------- Phase B: scores + exact top-64 penalty ----------------
        psA_cm.__exit__(None, None, None)
        psB_cm = tc.tile_pool(name="psB", bufs=1, space="PSUM")
        psB = psB_cm.__enter__()
        evB_cm = tc.tile_pool(name="evB", bufs=2, side="right")
        evB = evB_cm.__enter__()
        scores_t = evB.tile([128, S], f32, name="scores_t", bufs=1)
        for tt in range(8):
            tsl = slice(128 * tt, 128 * (tt + 1))
            for sc_ in range(2):
                ssl = slice(512 * sc_, 512 * (sc_ + 1))
                dps = [psB.tile([128, 512], f32, name=f"dp{hh}") for hh in range(IH)]
                for hh in range(IH):
                    nc.tensor.matmul(dps[hh][:], qTi[32 * hh:32 * (hh + 1), tsl],
                                     krep[32 * hh:32 * (hh + 1), ssl],
                                     start=True, stop=True, tile_position=(32 * hh, 0))
                terms = [evB.tile([128, 512], f32, name=f"term{hh}") for hh in range(IH)]
                for hh in range(IH):
                    nc.vector.tensor_scalar(terms[hh][:], dps[hh][:], 0.0,
                                            wsb[tt][:, hh:hh + 1],
                                            op0=AL.max, op1=AL.mult)
                acc = scores_t[:, ssl]
                nc.vector.tensor_tensor(out=acc, in0=terms[0][:], in1=terms[1][:], op=AL.add)
                nc.vector.tensor_tensor(out=acc, in0=acc, in1=terms[2][:], op=AL.add)
                nc.vector.tensor_tensor(out=acc, in0=acc, in1=terms[3][:], op=AL.add)
            # exact top-64: 8 rounds of max8 + match_replace (replaces lowest
            # index first on ties, matching lax.top_k)
            work = evB.tile([128, S], f32, name="work", bufs=1)
            nc.vector.tensor_copy(work[:], scores_t[:])
            m8 = evB.tile([128, 8], f32, name="m8")
            for r in range(8):
                nc.vector.max(out=m8[:], in_=work[:])
                nc.vector.match_replace(out=work[:], in_to_replace=m8[:],
                                        in_values=work[:], imm_value=NEG_FILL)
            # penalty [t, s]: 0 where selected (== NEG_FILL), -PEN elsewhere
            Pf = scores_t  # scores are dead after the copy into work; reuse the slot
            nc.vector.tensor_scalar(Pf[:], work[:], -1e29, -PEN,
                                    op0=AL.is_gt, op1=AL.mult)
            # transpose to [s, t] via PE, evacuate to bf16
            for sb in range(8):
                ptp = psB.tile([128, 128], f32, name="ptp", bufs=2)
                nc.tensor.matmul(ptp[:], Pf[:, 128 * sb:128 * (sb + 1)], identf[:],
                                 is_transpose=True, start=True, stop=True)
                nc.scalar.copy(PT[sb][:, tsl], ptp[:])

        # ---------------- Phase C: in_proj ----------------
        psB_cm.__exit__(None, None, None)
        psC_cm = tc.tile_pool(name="psC", bufs=1, space="PSUM")
        psC = psC_cm.__enter__()
        for m in range(8):  # q rows 0..511 (scaled 1/8), k rows 512..1023
            for tcol in range(2):
                sl = slice(512 * tcol, 512 * (tcol + 1))
                pqk = psC.tile([128, 512], f32, name="pqk", bufs=2)
                for kk in range(KD):
                    nc.tensor.matmul(pqk[:], ipw[kk][:, 128 * m:128 * (m + 1)],
                                     xTb[kk][:, sl], start=(kk == 0), stop=(kk == KD - 1))
                if m < 4:
                    nc.vector.tensor_scalar(qkT[m][:, sl], pqk[:], ipbqk[m][:], 0.125,
                                            op0=AL.add, op1=AL.mult)
                else:
                    nc.vector.tensor_scalar(qkT[m][:, sl], pqk[:], ipbqk[m][:], None,
                                            op0=AL.add)
        for sv in range(8):  # v in normal orientation [s, d_v] + ones col
            pv = psC.tile([128, 512], f32, name="pv", bufs=2)
            svl = slice(128 * sv, 128 * (sv + 1))
            for kk in range(KD):
                nc.tensor.matmul(pv[:], xTb[kk][:, svl], ipw[kk][:, 2 * D:3 * D],
                                 start=(kk == 0), stop=False)
            nc.tensor.matmul(pv[:], ones_row[:, svl], ipw_ones[:, 2 * D:3 * D],
                             start=False, stop=True)
            vv = vsb[sv][:].rearrange("p (h c) -> p h c", h=8)[:, :, 0:HD]
            nc.scalar.copy(vv, pv[:].rearrange("p (h c) -> p h c", h=8))
            for hh in range(8):
                nc.vector.memset(vsb[sv][:, (HD + 1) * hh + HD:(HD + 1) * (hh + 1)], 1.0)

        # ---------------- Phase D: attention per head ----------------
        psC_cm.__exit__(None, None, None)
        evB_cm.__exit__(None, None, None)
        poolB_cm.__exit__(None, None, None)
        psL_cm = tc.tile_pool(name="psL", bufs=2, space="PSUM")
        psL = psL_cm.__enter__()
        psV_cm = tc.tile_pool(name="psV", bufs=2, space="PSUM")
        psV = psV_cm.__enter__()
        for h in range(H):
            qrow = qkT[h // 2]
            krow = qkT[4 + h // 2]
            hb = 64 * (h % 2)
            for tcol in range(2):
                tl = slice(512 * tcol, 512 * (tcol + 1))
                pav = psV.tile([HD + 1, 512], f32, name="pav")
                for sv in range(8):
                    svl = slice(128 * sv, 128 * (sv + 1))
                    plg = psL.tile([128, 512], f32, name="pp")
                    # penalty first (4 identity-matmul adds), then logits
                    for qb in range(4):
                        nc.tensor.matmul(plg[:, 128 * qb:128 * (qb + 1)], identb[:],
                                         PT[sv][:, 512 * tcol + 128 * qb:512 * tcol + 128 * (qb + 1)],
                                         start=(qb == 0), stop=False)
                    nc.tensor.matmul(plg[:], krow[hb:hb + 64, svl], qrow[hb:hb + 64, tl],
                                     start=False, stop=True)
                    attu = evD.tile([128, 512], bf16, name="attu", bufs=2)
                    nc.scalar.activation(attu[:], plg[:], AF.Exp)
                    nc.tensor.matmul(pav[:], vsb[sv][:, (HD + 1) * h:(HD + 1) * (h + 1)],
                                     attu[:], start=(sv == 0), stop=(sv == 7))
                # normalize: rows 0..63 / row 64
                rec = evD.tile([1, 512], f32, name="rec", bufs=1)
                nc.vector.reciprocal(rec[:], pav[HD:HD + 1, :])
                pbc = psL.tile([64, 512], f32, name="pbc", bufs=1)
                nc.tensor.matmul(pbc[:], ones1f[:, 0:64], rec[:], start=True, stop=True)
                rbs = evD.tile([64, 512], f32, name="rbs", bufs=1)
                nc.scalar.copy(rbs[:], pbc[:])
                nc.vector.tensor_tensor(out=aoT[h // 2][hb:hb + 64, tl],
                                        in0=pav[0:HD, :], in1=rbs[:], op=AL.mult)

        poolX2_cm.__exit__(None, None, None)
        # ---------------- Phase E: out proj + residual + LN1 ----------------
        def layer_norm_T(src_tiles, dst_f32, dst_bf16, gamma, beta):
            # src [4][128, S] f32 feature-major; normalize over features
            # (partitions); optional per-feature affine applied at the end.
            sq = [evD.tile([128, S], bf16, name=f"sq{i}", bufs=1) for i in range(KD)]
            srcb = [evD.tile([128, S], bf16, name=f"srcb{i}", bufs=1) for i in range(KD)]
            for i in range(KD):
                nc.vector.tensor_copy(srcb[i][:], src_tiles[i][:])
                nc.scalar.square(sq[i][:], srcb[i][:])
            rows = evD.tile([97, S], f32, name="lnrows", bufs=1)
            st = evD.tile([128, 64], f32, name="lnst", bufs=1)
            for tcol in range(2):
                tl = slice(512 * tcol, 512 * (tcol + 1))
                pmu = psL.tile([1, 512], f32, name="pmu", bufs=1)
                psq = psL.tile([1, 512], f32, name="psq", bufs=1)
                for i in range(KD):
                    nc.tensor.matmul(pmu[:], onescol[:], srcb[i][:, tl],
                                     start=(i == 0), stop=(i == KD - 1))
                for i in range(KD):
                    nc.tensor.matmul(psq[:], onescol[:], sq[i][:, tl],
                                     start=(i == 0), stop=(i == KD - 1))
                nc.scalar.copy(rows[0:1, tl], pmu[:])
                nc.scalar.copy(rows[32:33, tl], psq[:])
            # reshape rows -> [128, 8] lanes, do the scalar math wide
            nc.sync.dma_start(st[:, 0:8], rows[0:1, :])
            nc.sync.dma_start(st[:, 8:16], rows[32:33, :])
            nc.vector.tensor_scalar(st[:, 16:24], st[:, 0:8], 1.0 / D, None, op0=AL.mult)   # mu
            nc.vector.tensor_scalar(st[:, 24:32], st[:, 8:16], 1.0 / D, None, op0=AL.mult)  # m2
            nc.vector.tensor_tensor(out=st[:, 32:40], in0=st[:, 16:24], in1=st[:, 16:24], op=AL.mult)
            nc.vector.tensor_tensor(out=st[:, 32:40], in0=st[:, 24:32], in1=st[:, 32:40], op=AL.subtract)
            nc.vector.tensor_scalar(st[:, 32:40], st[:, 32:40], EPS, None, op0=AL.add)      # var+eps
            nc.scalar.sqrt(st[:, 40:48], st[:, 32:40])
            nc.vector.reciprocal(st[:, 48:56], st[:, 40:48])                                 # rstd
            nc.vector.tensor_tensor(out=st[:, 56:64], in0=st[:, 16:24], in1=st[:, 48:56], op=AL.mult)  # msh
            nc.sync.dma_start(rows[64:65, :], st[:, 48:56])
            nc.sync.dma_start(rows[0:1, :], st[:, 56:64])  # musum slot is free by now
            rstdb = evD.tile([128, S], f32, name="rstdb", bufs=1)
            mshb = evD.tile([128, S], f32, name="mshb", bufs=1)
            for tcol in range(2):
                tl = slice(512 * tcol, 512 * (tcol + 1))
                pb1 = psL.tile([128, 512], f32, name="pmu", bufs=1)
                pb2 = psL.tile([128, 512], f32, name="psq", bufs=1)
                nc.tensor.matmul(pb1[:], onesbig[64:65, :], rows[64:65, tl], start=True, stop=True)
                nc.tensor.matmul(pb2[:], ones1f[:], rows[0:1, tl], start=True, stop=True)
                nc.scalar.copy(rstdb[:, tl], pb1[:])
                nc.scalar.copy(mshb[:, tl], pb2[:])
            for i in range(KD):
                t1 = evD.tile([128, S], f32, name="t1", bufs=1)
                nc.vector.tensor_tensor(out=t1[:], in0=src_tiles[i][:], in1=rstdb[:], op=AL.mult)
                nc.vector.tensor_tensor(out=t1[:], in0=t1[:], in1=mshb[:], op=AL.subtract)
                nc.vector.tensor_scalar(dst_f32[i][:], t1[:], gamma[i][:], beta[i][:],
                                        op0=AL.mult, op1=AL.add)
                if dst_bf16 is not None:
                    nc.vector.tensor_copy(dst_bf16[i][:], dst_f32[i][:])

        poolF_cm = tc.tile_pool(name="poolF", bufs=1)
        poolF = poolF_cm.__enter__()
        poolF1_cm = tc.tile_pool(name="poolF1", bufs=1)
        poolF1 = poolF1_cm.__enter__()
        hTf = [poolF.tile([128, S], f32, name=f"hTf{i}") for i in range(KD)]
        hTb = [poolF.tile([128, S], bf16, name=f"hTb{i}") for i in range(KD)]
        gsb = [poolF.tile([128, 512], bf16, name=f"gsb{i}") for i in range(16)]
        r1 = [poolF1.tile([128, S], f32, name=f"r1_{i}") for i in range(KD)]
        for m in range(KD):
            for tcol in range(2):
                tl = slice(512 * tcol, 512 * (tcol + 1))
                po = psL.tile([128, 512], f32, name="pp")
                for kk in range(KD):
                    nc.tensor.matmul(po[:], outw[kk][:, 128 * m:128 * (m + 1)],
                                     aoT[kk][:, tl], start=(kk == 0), stop=(kk == KD - 1))
                nc.vector.scalar_tensor_tensor(out=r1[m][:, tl], in0=po[:], scalar=outb[m][:],
                                               in1=xTf[m][:, tl], op0=AL.add, op1=AL.add)
        poolX_cm.__exit__(None, None, None)
        layer_norm_T(r1, hTf, hTb, n1g, n1b)
        poolF1_cm.__exit__(None, None, None)

        # ---------------- Phase F: FFN + residual + LN2 ----------------
        r2 = [poolF.tile([128, S], f32, name=f"r2_{i}") for i in range(KD)]
        for tcol in range(2):
            tl = slice(512 * tcol, 512 * (tcol + 1))
            for n in range(16):
                pf = psL.tile([128, 512], f32, name="pp")
                for kk in range(KD):
                    nc.tensor.matmul(pf[:], l1w[kk][:, 128 * n:128 * (n + 1)],
                                     hTb[kk][:, tl], start=(kk == 0), stop=(kk == KD - 1))
                nc.scalar.activation(gsb[n][:], pf[:], AF.Gelu, bias=l1b[n][:])
            for m in range(KD):
                p2 = psL.tile([128, 512], f32, name="pp")
                for kk in range(16):
                    nc.tensor.matmul(p2[:], l2w[kk][:, 128 * m:128 * (m + 1)],
                                     gsb[kk][:], start=(kk == 0), stop=(kk == 15))
                nc.vector.scalar_tensor_tensor(out=r2[m][:, tl], in0=p2[:], scalar=l2b[m][:],
                                               in1=hTf[m][:, tl], op0=AL.add, op1=AL.add)
        yT = [poolF.tile([128, S], f32, name=f"yT{i}") for i in range(KD)]
        layer_norm_T(r2, yT, None, n2g, n2b)
        # transpose back to token-major and store
        for t in range(8):
            ysd = poolF.tile([128, D], f16, name="ysd", bufs=2)
            ptile = psL.tile([128, 512], f32, name="pp")
            for i in range(KD):
                nc.tensor.matmul(ptile[:, 128 * i:128 * (i + 1)],
                                 yT[i][:, 128 * t:128 * (t + 1)], identf[:],
                                 is_transpose=True, start=True, stop=True)
            nc.scalar.copy(ysd[:], ptile[:])
            nc.sync.dma_start(d_y[128 * t:128 * (t + 1), :], ysd[:])

        poolF_cm.__exit__(None, None, None)
        evD_cm.__exit__(None, None, None)
        psV_cm.__exit__(None, None, None)
        psL_cm.__exit__(None, None, None)
        wpool_cm.__exit__(None, None, None)

    legalize_waits(nc, 1)
    return nc


# --- cached PJRT dispatch ---------------------------------------------------
#
# run_bass_kernel_spmd rebuilds a fresh jax.jit wrapper (and reloads the NEFF,
# and re-uploads every weight 8x) on every call. This replicates its exact
# lowering path (_bass_exec_p under jit(shard_map(...))) but compiles once and
# keeps the weights device-resident, so steady-state calls only move x and y.

_CTX = {}
_POOL = ThreadPoolExecutor(8)


def _cast_par(src, dst):
    """dst[...] = src with dtype conversion, split across threads."""
    n = src.shape[0]
    step = (n + 7) // 8
    def seg(i):
        dst[i:i + step] = src[i:i + step]
    list(_POOL.map(seg, range(0, n, step)))
    return dst


def _get_ctx():
    if "jitted" in _CTX:
        return _CTX

    nc = build_kernel()
    _b2j.install_neuronx_cc_hook()

    partition_name = nc.partition_id_tensor.name if nc.partition_id_tensor else None
    in_names, out_names, out_avals = [], [], []
    for alloc in nc.m.functions[0].allocations:
        if not isinstance(alloc, mybir.MemoryLocationSet):
            continue
        name = alloc.memorylocations[0].name
        if alloc.kind == "ExternalInput":
            if name != partition_name:
                in_names.append(name)
        elif alloc.kind == "ExternalOutput":
            out_names.append(name)
            out_avals.append(jax.core.ShapedArray(tuple(alloc.tensor_shape),
                                                  mybir.dt.np(alloc.dtype)))
    n_params = len(in_names)
    all_names = in_names + out_names  # zero output buffers ride as extra inputs
    bind_names = all_names + ([partition_name] if partition_name else [])

    def _body(*args):
        operands = list(args)
        if partition_name is not None:
            operands.append(_b2j.partition_id_tensor())
        outs = _b2j._bass_exec_p.bind(
            *operands,
            out_avals=tuple(out_avals),
            in_names=tuple(bind_names),
            out_names=tuple(out_names),
            lowering_input_output_aliases=(),
            sim_require_finite=True,
            sim_require_nnan=True,
            nc=nc,
        )
        return tuple(outs)

    devices = jax.devices()[:B]
    mesh = Mesh(np.asarray(devices), ("core",))
    # x and y are column shards of [S, B*D]; everything else is replicated.
    sharded_names = {"xin": P(None, "core"), "y": P(None, "core")}
    in_specs = tuple(sharded_names.get(n, P()) for n in all_names)
    out_specs = (P(None, "core"),) * len(out_names)

    def make_jit():
        return jax.jit(
            shard_map(_body, mesh=mesh, in_specs=in_specs, out_specs=out_specs,
                      check_rep=False),
            keep_unused=True,
        )

    jitted = make_jit()

    # AOT-compile a second copy with the bass effect suppressed so steady
    # state dispatch takes the C++ fast path; fall back to `jitted` if the
    # compiled callable rejects our argument mix.
    shapes = {}
    for alloc in nc.m.functions[0].allocations:
        if isinstance(alloc, mybir.MemoryLocationSet) and alloc.tensor_shape:
            shapes[alloc.memorylocations[0].name] = (
                tuple(alloc.tensor_shape), mybir.dt.np(alloc.dtype))
    sds = []
    for n in all_names:
        shape, dt = shapes[n]
        spec = sharded_names.get(n, P())
        gshape = tuple(
            s * (B if i < len(spec) and spec[i] == "core" else 1)
            for i, s in enumerate(shape))
        sds.append(jax.ShapeDtypeStruct(gshape, dt,
                                        sharding=NamedSharding(mesh, spec)))
    try:
        compiled = _b2j.fast_dispatch_compile(
            lambda: make_jit().lower(*sds).compile())
    except Exception:
        compiled = None

    _CTX.update(nc=nc, mesh=mesh, jitted=jitted, compiled=compiled,
                in_names=in_names, out_names=out_names, n_params=n_params)
    return _CTX


def _prep_weights(ctx, key, iq_w, iq_b, ik_w, ik_b, iw_w, iw_b, in_proj_w,
                  in_proj_b, out_w, out_b, l1_w, l1_b, l2_w, l2_b,
                  n1_g, n1_b, n2_g, n2_b):
    bf = ml_dtypes.bfloat16
    ik4 = np.tile(ik_w, (1, 4)).astype(np.float32)            # [D, 128]
    ik4b = np.tile(ik_b, 4)[:, None].astype(np.float32)        # [128, 1]
    iwb_rep = np.tile(iw_b[None, :], (128, 1)).astype(np.float32)
    ipw_ext = np.zeros((D + 1, 3 * D), np.float32)
    ipw_ext[:D] = in_proj_w
    ipw_ext[D, 2 * D:] = in_proj_b[2 * D:]
    ident = np.eye(128, dtype=np.float32)
    host = {
        "iqw": iq_w, "iqb": iq_b[:, None], "ik4": ik4, "ik4b": ik4b,
        "iww": iw_w, "iwb": iwb_rep,
        "ipw": ipw_ext.astype(bf), "ipbqk": in_proj_b[:2 * D][:, None],
        "outw": out_w.astype(bf), "outb": out_b[:, None],
        "l1w": l1_w.astype(bf), "l1b": l1_b[:, None],
        "l2w": l2_w.astype(bf), "l2b": l2_b[:, None],
        "n1g": n1_g[:, None], "n1b": n1_b[:, None],
        "n2g": n2_g[:, None], "n2b": n2_b[:, None],
        "idf": ident, "idbf": ident.astype(bf), "idh": ident.astype(np.float16),
    }
    rep = NamedSharding(ctx["mesh"], P())
    dev = {k: jax.device_put(np.ascontiguousarray(v), rep) for k, v in host.items()}
    # zero buffer the ExternalOutput rides in on (kernel writes every element)
    dev["y"] = jax.device_put(np.zeros((S, B * D), np.float16),
                              NamedSharding(ctx["mesh"], P(None, "core")))
    ctx["dev_args"] = dev
    ctx["weights_key"] = key


def kernel(x, iq_w, iq_b, ik_w, ik_b, iw_w, iw_b, in_proj_w, in_proj_b,
           out_w, out_b, l1_w, l1_b, l2_w, l2_b, n1_g, n1_b, n2_g, n2_b):
    ctx = _get_ctx()
    weights = (iq_w, iq_b, ik_w, ik_b, iw_w, iw_b, in_proj_w, in_proj_b,
               out_w, out_b, l1_w, l1_b, l2_w, l2_b, n1_g, n1_b, n2_g, n2_b)
    key = tuple(id(w) for w in weights)
    if ctx.get("weights_key") != key:
        f = lambda a: np.asarray(a, np.float32)
        _prep_weights(ctx, key, *(f(w) for w in weights))

    xv = np.asarray(x).reshape(S, B * D)
    if xv.dtype == np.float16:
        xg = xv
    else:
        xg = _cast_par(xv, np.empty((S, B * D), np.float16))
    arg_map = {**ctx["dev_args"], "xin": xg}
    args = [arg_map[n] for n in ctx["in_names"] + ctx["out_names"]]
    fn = ctx.get("compiled")
    if fn is not None:
        try:
            outs = fn(*args)
        except Exception:
            ctx["compiled"] = None
            outs = ctx["jitted"](*args)
    else:
        outs = ctx["jitted"](*args)
    y16 = np.asarray(outs[0])
    y32 = _cast_par(y16, np.empty((S, B * D), np.float32))
    return y32.reshape(S, B, D)
